# revision 11
# baseline (speedup 1.0000x reference)
"""Multi-head attention (B=2,S=2048,D=1024,H=16,hd=64) on 8 TRN2 cores.

Head-sharded tensor parallel per core: core c owns heads (2c, 2c+1).
  1. qk^T projection -> Q^T/K^T in [dim, token] layout (bf16)
  2. V projection    -> V in [token, dim] layout, ones-augmented (bf16)
  3. logits^T = K Q^T per 128-key tile -> PSUM, exp via ACT (scale=1/8) -> P bf16
  4. vals^T_aug = V_aug^T @ P accumulated in PSUM; row 64 = softmax denom Z
  5. normalize via ones-matmul broadcast of Z + DVE divide
  6. AllToAll so core c ends with full-feature vals^T for its 512-token slice
  7. o_proj (f32r full-rate matmuls) -> out slice [512, 1024]
Host concatenates the 8 token slices.
"""

import numpy as np
import ml_dtypes

import concourse.bass as bass
import concourse.mybir as mybir
from concourse import bacc
from concourse import tile
from concourse.bass_utils import run_bass_kernel_spmd

F32 = mybir.dt.float32
F32R = mybir.dt.float32r
BF16 = mybir.dt.bfloat16
EXP = mybir.ActivationFunctionType.Exp

B, S, D, E, H = 2, 2048, 1024, 1024, 16
HD = 64           # head dim
T = B * S         # 4096 tokens
NC = 8            # cores
TSL = T // NC     # 512 tokens per core for o_proj


def build_nc():
    nc = bacc.Bacc("TRN2", target_bir_lowering=False, debug=False)

    xT = nc.dram_tensor("xT", [D, T], BF16, kind="ExternalInput")
    wqkT = nc.dram_tensor("wqkT", [D, 256], BF16, kind="ExternalInput")
    wvT = nc.dram_tensor("wvT", [D, 128], BF16, kind="ExternalInput")
    bqk = nc.dram_tensor("bqk", [128, 2], F32, kind="ExternalInput")
    bv = nc.dram_tensor("bv", [1, 128], BF16, kind="ExternalInput")
    woT = nc.dram_tensor("woT", [D, E], F32R, kind="ExternalInput")
    bo = nc.dram_tensor("bo", [1, E], F32R, kind="ExternalInput")
    out = nc.dram_tensor("out", [TSL, E], F32, kind="ExternalOutput")

    with tile.TileContext(nc, num_cores=NC) as tc:
        with (
            tc.tile_pool(name="pers", bufs=1) as pers,
            tc.tile_pool(name="work", bufs=2) as work,
            tc.tile_pool(name="ps", bufs=2, space="PSUM") as ps,
            tc.tile_pool(name="dram", bufs=1, space="DRAM") as dram,
        ):
            # ---- persistent SBUF ----
            q_sb = pers.tile([128, T], BF16, tag="q")      # rows 0-63 h0, 64-127 h1
            k_sb = pers.tile([128, T], BF16, tag="k")
            vals0 = pers.tile([64, T], F32, tag="vals0")   # normalized valsT head0
            vals1 = pers.tile([64, T], F32, tag="vals1")
            wqk_sb = [pers.tile([128, 256], BF16, tag=f"wqk{i}", name=f"wqk{i}") for i in range(8)]
            wv_sb = [pers.tile([128, 128], BF16, tag=f"wv{i}", name=f"wv{i}") for i in range(8)]
            wo_sb = [pers.tile([128, E], F32R, tag=f"wo{i}", name=f"wo{i}") for i in range(8)]
            bqk_sb = pers.tile([128, 2], F32, tag="bqk")
            bv_sb = pers.tile([1, 128], BF16, tag="bv")
            bo_sb = pers.tile([1, E], F32R, tag="bo")
            ones_bf = pers.tile([1, 128], BF16, tag="onesbf")
            ones_f32 = pers.tile([128, 128], F32, tag="onesf32")
            ones_f = pers.tile([128, 128], F32R, tag="onesf")
            vaug = [pers.tile([128, 130], BF16, tag=f"vg{i}", name=f"vg{i}") for i in range(32)]

            nc.vector.memset(ones_bf[:, :], 1.0)
            nc.vector.memset(ones_f32[:, :], 1.0)
            nc.vector.tensor_copy(out=ones_f[:, :], in_=ones_f32[:, :])
            for i in range(32):
                nc.vector.memset(vaug[i][:, 64:65], 1.0)
                nc.vector.memset(vaug[i][:, 129:130], 1.0)

            nc.sync.dma_start(out=bqk_sb[:, :], in_=bqk[:, :])
            nc.sync.dma_start(out=bv_sb[:, :], in_=bv[:, :])
            nc.sync.dma_start(out=bo_sb[:, :], in_=bo[:, :])
            for i in range(8):
                nc.sync.dma_start(out=wqk_sb[i][:, :], in_=wqkT[i * 128:(i + 1) * 128, :])
                nc.sync.dma_start(out=wv_sb[i][:, :], in_=wvT[i * 128:(i + 1) * 128, :])

            # xt streamed in 4 token-blocks of 1024
            xt = {}

            def load_block(tb):
                for kt in range(8):
                    t_ = work.tile([128, 1024], BF16, tag=f"xt{kt}", bufs=2,
                                   name=f"xt{kt}_{tb}")
                    nc.sync.dma_start(
                        out=t_[:, :],
                        in_=xT[kt * 128:(kt + 1) * 128, tb * 1024:(tb + 1) * 1024])
                    xt[(tb, kt)] = t_

            def proj_block(tb):
                # qk projection: out rows 0-255, tokens tb*1024..+1024
                for mt in range(2):
                    acc = ps.tile([128, 1024], F32, tag="lg", name=f"qkp{tb}{mt}")
                    for kt in range(8):
                        for nb in range(2):
                            nc.tensor.matmul(
                                acc[:, nb * 512:(nb + 1) * 512],
                                lhsT=wqk_sb[kt][:, mt * 128:(mt + 1) * 128],
                                rhs=xt[(tb, kt)][:, nb * 512:(nb + 1) * 512],
                                start=(kt == 0), stop=(kt == 7))
                    dst = q_sb if mt == 0 else k_sb
                    nc.vector.tensor_scalar(
                        out=dst[:, tb * 1024:(tb + 1) * 1024], in0=acc[:, :],
                        scalar1=bqk_sb[:, mt:mt + 1], scalar2=None,
                        op0=mybir.AluOpType.add)
                # v projection: token tiles tb*8 .. tb*8+8
                for vi in range(8):
                    ti = tb * 8 + vi
                    vp = ps.tile([128, 128], F32, tag="lg", name=f"vp{ti}")
                    for kt in range(8):
                        nc.tensor.matmul(
                            vp[:, :],
                            lhsT=xt[(tb, kt)][:, vi * 128:(vi + 1) * 128],
                            rhs=wv_sb[kt][:, :],
                            start=(kt == 0), stop=False)
                    nc.tensor.matmul(vp[:, :], lhsT=ones_bf[:, :],
                                     rhs=bv_sb[:, :], start=False, stop=True)
                    nc.vector.tensor_copy(out=vaug[ti][:, 0:64], in_=vp[:, 0:64])
                    nc.vector.tensor_copy(out=vaug[ti][:, 65:129], in_=vp[:, 64:128])

            def attention(b, qh):
                """heads packed in PE rows; q-half of 1024 columns."""
                q0 = b * 2048 + qh * 1024
                vt = {}
                for h in range(2):
                    vt[h] = ps.tile([65, 1024], F32, tag="vt", name=f"vt{b}{qh}{h}")
                for kt in range(16):
                    pt = {}
                    for h in range(2):
                        lg = ps.tile([128, 1024], F32, tag="lg", name=f"lg{b}{qh}{kt}{h}")
                        for nb in range(2):
                            nc.tensor.matmul(
                                lg[:, nb * 512:(nb + 1) * 512],
                                lhsT=k_sb[h * 64:(h + 1) * 64,
                                          b * 2048 + kt * 128: b * 2048 + (kt + 1) * 128],
                                rhs=q_sb[h * 64:(h + 1) * 64,
                                         q0 + nb * 512: q0 + (nb + 1) * 512],
                                start=True, stop=True)
                        p = work.tile([128, 1024], BF16, tag="p", bufs=4,
                                      name=f"p{b}{qh}{kt}{h}")
                        nc.scalar.activation(p[:, :], lg[:, :], EXP, scale=0.125)
                        pt[h] = p
                    for h in range(2):
                        for nb in range(2):
                            nc.tensor.matmul(
                                vt[h][:, nb * 512:(nb + 1) * 512],
                                lhsT=vaug[b * 16 + kt][:, h * 65:(h + 1) * 65],
                                rhs=pt[h][:, nb * 512:(nb + 1) * 512],
                                start=(kt == 0), stop=(kt == 15))
                for h in range(2):
                    vu = work.tile([65, 1024], F32, tag="vu", bufs=2,
                                   name=f"vu{b}{qh}{h}")
                    nc.vector.tensor_copy(out=vu[:, :], in_=vt[h][:, :])
                    rz = work.tile([65, 1024], F32, tag="rz", bufs=2,
                                   name=f"rz{b}{qh}{h}")
                    nc.vector.reciprocal(out=rz[64:65, :], in_=vu[64:65, :])
                    zfr = work.tile([65, 1024], F32R, tag="zfr", bufs=2,
                                    name=f"zfr{b}{qh}{h}")
                    nc.vector.tensor_copy(out=zfr[64:65, :], in_=rz[64:65, :])
                    zb = ps.tile([64, 1024], F32, tag="lg", name=f"zb{b}{qh}{h}")
                    for nb in range(2):
                        nc.tensor.matmul(
                            zb[:, nb * 512:(nb + 1) * 512],
                            lhsT=ones_f[64:65, 0:64],
                            rhs=zfr[64:65, nb * 512:(nb + 1) * 512],
                            start=True, stop=True)
                    dst = vals0 if h == 0 else vals1
                    nc.vector.tensor_tensor(
                        out=dst[:, q0:q0 + 1024], in0=vu[0:64, :], in1=zb[:, :],
                        op=mybir.AluOpType.mult)

            # ---- schedule ----
            load_block(0)
            load_block(1)
            for i in range(8):
                nc.sync.dma_start(out=wo_sb[i][:, :], in_=woT[i * 128:(i + 1) * 128, :])
            proj_block(0)
            proj_block(1)
            attention(0, 0)
            load_block(2)
            proj_block(2)
            attention(0, 1)
            load_block(3)
            proj_block(3)
            attention(1, 0)
            attention(1, 1)

            # ---- AllToAll: rank j gets token slice j with all features ----
            a2a_in = dram.tile([NC * 128, TSL], F32, tag="a2ain")
            a2a_out = dram.tile([NC * 128, TSL], F32, tag="a2aout")
            for j in range(NC):
                nc.sync.dma_start(out=a2a_in[j * 128: j * 128 + 64, :],
                                  in_=vals0[:, j * TSL:(j + 1) * TSL])
                nc.sync.dma_start(out=a2a_in[j * 128 + 64: (j + 1) * 128, :],
                                  in_=vals1[:, j * TSL:(j + 1) * TSL])
            nc.gpsimd.collective_compute(
                "AllToAll", mybir.AluOpType.bypass,
                replica_groups=[list(range(NC))],
                ins=[a2a_in.opt()], outs=[a2a_out.opt()])

            # ---- o_proj on my 512-token slice ----
            va = [work.tile([128, TSL], F32, tag=f"va{i}", bufs=1, name=f"va{i}")
          for i in range(8)]
            va_fr = [work.tile([128, TSL], F32R, tag=f"vafr{i}", bufs=1, name=f"vafr{i}")
                  for i in range(8)]
            for i in range(8):
                nc.sync.dma_start(out=va[i][:, :],
                                  in_=a2a_out[i * 128:(i + 1) * 128, :])
                nc.vector.tensor_copy(out=va_fr[i][:, :], in_=va[i][:, :])
            for mt in range(4):
                for nb in range(2):
                    op = ps.tile([128, 512], F32, tag="lg", name=f"op{mt}{nb}")
                    for kt in range(8):
                        nc.tensor.matmul(
                            op[:, :],
                            lhsT=va_fr[kt][:, mt * 128:(mt + 1) * 128],
                            rhs=wo_sb[kt][:, nb * 512:(nb + 1) * 512],
                            start=(kt == 0), stop=False)
                    nc.tensor.matmul(
                        op[:, :], lhsT=ones_f[0:1, 0:128],
                        rhs=bo_sb[:, nb * 512:(nb + 1) * 512],
                        start=False, stop=True)
                    ot = work.tile([128, 512], F32, tag="ot", bufs=3,
                                   name=f"ot{mt}{nb}")
                    nc.vector.tensor_copy(out=ot[:, :], in_=op[:, :])
                    nc.sync.dma_start(
                        out=out[mt * 128:(mt + 1) * 128, nb * 512:(nb + 1) * 512],
                        in_=ot[:, :])
    nc.compile()
    return nc


def _prep_inputs(x, Wqkv, bqkv, Wo, bo):
    x = np.asarray(x, np.float32)
    Wqkv = np.asarray(Wqkv, np.float32)
    bqkv = np.asarray(bqkv, np.float32)
    Wo = np.asarray(Wo, np.float32)
    bo = np.asarray(bo, np.float32)
    xT = np.ascontiguousarray(x.reshape(T, D).T).astype(ml_dtypes.bfloat16)
    woT = np.ascontiguousarray(Wo.T)
    bo2 = np.ascontiguousarray(bo.reshape(1, E))
    in_maps = []
    for c in range(NC):
        h0, h1 = 2 * c, 2 * c + 1
        qk_idx = np.concatenate([
            np.arange(h0 * 192, h0 * 192 + 64),
            np.arange(h1 * 192, h1 * 192 + 64),
            np.arange(h0 * 192 + 64, h0 * 192 + 128),
            np.arange(h1 * 192 + 64, h1 * 192 + 128)])
        v_idx = np.concatenate([
            np.arange(h0 * 192 + 128, h0 * 192 + 192),
            np.arange(h1 * 192 + 128, h1 * 192 + 192)])
        in_maps.append({
            "xT": xT,
            "wqkT": np.ascontiguousarray(Wqkv[qk_idx].T).astype(ml_dtypes.bfloat16),
            "wvT": np.ascontiguousarray(Wqkv[v_idx].T).astype(ml_dtypes.bfloat16),
            "bqk": np.ascontiguousarray(bqkv[qk_idx].reshape(2, 128).T),
            "bv": np.ascontiguousarray(bqkv[v_idx].reshape(1, 128)).astype(
                ml_dtypes.bfloat16),
            "woT": woT,
            "bo": bo2,
        })
    return in_maps


_NC_CACHE = {}


def run(x, Wqkv, bqkv, Wo, bo, trace=False):
    if "nc" not in _NC_CACHE:
        _NC_CACHE["nc"] = build_nc()
    nc = _NC_CACHE["nc"]
    in_maps = _prep_inputs(x, Wqkv, bqkv, Wo, bo)
    res = run_bass_kernel_spmd(nc, in_maps, core_ids=list(range(NC)), trace=trace)
    full = np.concatenate([res.results[c]["out"] for c in range(NC)], axis=0)
    return full, res


def kernel(x, Wqkv, bqkv, Wo, bo):
    full, _ = run(x, Wqkv, bqkv, Wo, bo, trace=False)
    return full


# revision 13
# speedup vs baseline: 1.4405x; 1.4405x over previous
"""Multi-head attention (B=2,S=2048,D=1024,H=16,hd=64) on 8 TRN2 cores.

Head-sharded tensor parallel per core: core c owns heads (2c, 2c+1).
  1. qk^T projection -> Q^T/K^T in [dim, token] layout (bf16)
  2. V projection    -> V in [token, dim] layout, ones-augmented (bf16)
  3. logits^T = K Q^T per 128-key tile -> PSUM, exp via ACT (scale=1/8) -> P bf16
  4. vals^T_aug = V_aug^T @ P accumulated in PSUM; row 64 = softmax denom Z
  5. normalize via ones-matmul broadcast of Z + DVE divide
  6. AllToAll so core c ends with full-feature vals^T for its 512-token slice
  7. o_proj (f32r full-rate matmuls) -> out slice [512, 1024]
Host concatenates the 8 token slices.
"""

import numpy as np
import ml_dtypes

import concourse.bass as bass
import concourse.mybir as mybir
from concourse import bacc
from concourse import tile
from concourse.bass_utils import run_bass_kernel_spmd

F32 = mybir.dt.float32
F32R = mybir.dt.float32r
BF16 = mybir.dt.bfloat16
EXP = mybir.ActivationFunctionType.Exp

B, S, D, E, H = 2, 2048, 1024, 1024, 16
HD = 64           # head dim
T = B * S         # 4096 tokens
NC = 8            # cores
TSL = T // NC     # 512 tokens per core for o_proj


def build_nc():
    nc = bacc.Bacc("TRN2", target_bir_lowering=False, debug=False)

    xT = nc.dram_tensor("xT", [D, T], BF16, kind="ExternalInput")
    wqkT = nc.dram_tensor("wqkT", [D, 256], BF16, kind="ExternalInput")
    wvT = nc.dram_tensor("wvT", [D, 128], BF16, kind="ExternalInput")
    bqk = nc.dram_tensor("bqk", [128, 2], F32, kind="ExternalInput")
    bv = nc.dram_tensor("bv", [1, 128], BF16, kind="ExternalInput")
    woT = nc.dram_tensor("woT", [D, E], F32R, kind="ExternalInput")
    bo = nc.dram_tensor("bo", [1, E], F32R, kind="ExternalInput")
    out = nc.dram_tensor("out", [TSL, E], F32, kind="ExternalOutput")

    with tile.TileContext(nc, num_cores=NC) as tc:
        with (
            tc.tile_pool(name="pers", bufs=1) as pers,
            tc.tile_pool(name="work", bufs=2) as work,
            tc.tile_pool(name="ps", bufs=2, space="PSUM") as ps,
            tc.tile_pool(name="dram", bufs=1, space="DRAM") as dram,
        ):
            # ---- persistent SBUF ----
            q_sb = pers.tile([128, T], BF16, tag="q")      # rows 0-63 h0, 64-127 h1
            k_sb = pers.tile([128, T], BF16, tag="k")
            vals0 = pers.tile([64, T], F32, tag="vals0")   # normalized valsT head0
            vals1 = pers.tile([64, T], F32, tag="vals1")
            wqk_sb = [pers.tile([128, 256], BF16, tag=f"wqk{i}", name=f"wqk{i}") for i in range(8)]
            wv_sb = [pers.tile([128, 128], BF16, tag=f"wv{i}", name=f"wv{i}") for i in range(8)]
            wo_sb = [pers.tile([128, E], F32R, tag=f"wo{i}", name=f"wo{i}") for i in range(8)]
            bqk_sb = pers.tile([128, 2], F32, tag="bqk")
            bv_sb = pers.tile([1, 128], BF16, tag="bv")
            bo_sb = pers.tile([1, E], F32R, tag="bo")
            ones_bf = pers.tile([1, 128], BF16, tag="onesbf")
            ones_f32 = pers.tile([128, 128], F32, tag="onesf32")
            ones_f = pers.tile([128, 128], F32R, tag="onesf")
            vaug = [pers.tile([128, 130], BF16, tag=f"vg{i}", name=f"vg{i}") for i in range(32)]

            nc.vector.memset(ones_bf[:, :], 1.0)
            nc.vector.memset(ones_f32[:, :], 1.0)
            nc.vector.tensor_copy(out=ones_f[:, :], in_=ones_f32[:, :])
            for i in range(32):
                nc.vector.memset(vaug[i][:, 64:65], 1.0)
                nc.vector.memset(vaug[i][:, 129:130], 1.0)

            nc.sync.dma_start(out=bqk_sb[:, :], in_=bqk[:, :])
            nc.sync.dma_start(out=bv_sb[:, :], in_=bv[:, :])
            nc.sync.dma_start(out=bo_sb[:, :], in_=bo[:, :])
            for i in range(8):
                nc.sync.dma_start(out=wqk_sb[i][:, :], in_=wqkT[i * 128:(i + 1) * 128, :])
                nc.sync.dma_start(out=wv_sb[i][:, :], in_=wvT[i * 128:(i + 1) * 128, :])

            # xt streamed in 4 token-blocks of 1024
            xt = {}

            def load_block(tb):
                for kt in range(8):
                    t_ = work.tile([128, 1024], BF16, tag=f"xt{kt}", bufs=2,
                                   name=f"xt{kt}_{tb}")
                    nc.sync.dma_start(
                        out=t_[:, :],
                        in_=xT[kt * 128:(kt + 1) * 128, tb * 1024:(tb + 1) * 1024])
                    xt[(tb, kt)] = t_

            def proj_block(tb):
                # qk projection: out rows 0-255, tokens tb*1024..+1024
                for mt in range(2):
                    acc = ps.tile([128, 1024], F32, tag="lg", name=f"qkp{tb}{mt}")
                    for kt in range(8):
                        for nb in range(2):
                            nc.tensor.matmul(
                                acc[:, nb * 512:(nb + 1) * 512],
                                lhsT=wqk_sb[kt][:, mt * 128:(mt + 1) * 128],
                                rhs=xt[(tb, kt)][:, nb * 512:(nb + 1) * 512],
                                start=(kt == 0), stop=(kt == 7))
                    dst = q_sb if mt == 0 else k_sb
                    nc.vector.tensor_scalar(
                        out=dst[:, tb * 1024:(tb + 1) * 1024], in0=acc[:, :],
                        scalar1=bqk_sb[:, mt:mt + 1], scalar2=None,
                        op0=mybir.AluOpType.add)
                # v projection: token tiles tb*8 .. tb*8+8
                for vi in range(8):
                    ti = tb * 8 + vi
                    vp = ps.tile([128, 128], F32, tag="lg", name=f"vp{ti}")
                    for kt in range(8):
                        nc.tensor.matmul(
                            vp[:, :],
                            lhsT=xt[(tb, kt)][:, vi * 128:(vi + 1) * 128],
                            rhs=wv_sb[kt][:, :],
                            start=(kt == 0), stop=False)
                    nc.tensor.matmul(vp[:, :], lhsT=ones_bf[:, :],
                                     rhs=bv_sb[:, :], start=False, stop=True)
                    nc.vector.tensor_copy(out=vaug[ti][:, 0:64], in_=vp[:, 0:64])
                    nc.vector.tensor_copy(out=vaug[ti][:, 65:129], in_=vp[:, 64:128])

            def attention(b, qh):
                """heads packed in PE rows; q-half of 1024 columns."""
                q0 = b * 2048 + qh * 1024
                vt = {}
                for h in range(2):
                    vt[h] = ps.tile([65, 1024], F32, tag="vt", name=f"vt{b}{qh}{h}")
                for kt in range(16):
                    pt = {}
                    for h in range(2):
                        lg = ps.tile([128, 1024], F32, tag="lg", name=f"lg{b}{qh}{kt}{h}")
                        for nb in range(2):
                            nc.tensor.matmul(
                                lg[:, nb * 512:(nb + 1) * 512],
                                lhsT=k_sb[h * 64:(h + 1) * 64,
                                          b * 2048 + kt * 128: b * 2048 + (kt + 1) * 128],
                                rhs=q_sb[h * 64:(h + 1) * 64,
                                         q0 + nb * 512: q0 + (nb + 1) * 512],
                                start=True, stop=True)
                        p = work.tile([128, 1024], BF16, tag="p", bufs=4,
                                      name=f"p{b}{qh}{kt}{h}")
                        nc.scalar.activation(p[:, :], lg[:, :], EXP, scale=0.125)
                        pt[h] = p
                    for h in range(2):
                        for nb in range(2):
                            nc.tensor.matmul(
                                vt[h][:, nb * 512:(nb + 1) * 512],
                                lhsT=vaug[b * 16 + kt][:, h * 65:(h + 1) * 65],
                                rhs=pt[h][:, nb * 512:(nb + 1) * 512],
                                start=(kt == 0), stop=(kt == 15))
                for h in range(2):
                    vu = work.tile([65, 1024], F32, tag="vu", bufs=2,
                                   name=f"vu{b}{qh}{h}")
                    nc.vector.tensor_copy(out=vu[:, :], in_=vt[h][:, :])
                    rz = work.tile([65, 1024], F32, tag="rz", bufs=2,
                                   name=f"rz{b}{qh}{h}")
                    nc.vector.reciprocal(out=rz[64:65, :], in_=vu[64:65, :])
                    zfr = work.tile([65, 1024], F32R, tag="zfr", bufs=2,
                                    name=f"zfr{b}{qh}{h}")
                    nc.vector.tensor_copy(out=zfr[64:65, :], in_=rz[64:65, :])
                    zb = ps.tile([64, 1024], F32, tag="lg", name=f"zb{b}{qh}{h}")
                    for nb in range(2):
                        nc.tensor.matmul(
                            zb[:, nb * 512:(nb + 1) * 512],
                            lhsT=ones_f[64:65, 0:64],
                            rhs=zfr[64:65, nb * 512:(nb + 1) * 512],
                            start=True, stop=True)
                    dst = vals0 if h == 0 else vals1
                    nc.vector.tensor_tensor(
                        out=dst[:, q0:q0 + 1024], in0=vu[0:64, :], in1=zb[:, :],
                        op=mybir.AluOpType.mult)

            # ---- per-batch AllToAll + o_proj (b0 overlaps b1 attention) ----
            TSB = 256  # tokens per (core, batch)

            def tail(b):
                a2a_in = dram.tile([NC * 128, TSB], F32, tag=f"a2ain{b}",
                                   name=f"a2ain{b}")
                a2a_out = dram.tile([NC * 128, TSB], F32, tag=f"a2aout{b}",
                                    name=f"a2aout{b}")
                for j in range(NC):
                    c0 = b * 2048 + j * TSB
                    nc.sync.dma_start(out=a2a_in[j * 128: j * 128 + 64, :],
                                      in_=vals0[:, c0:c0 + TSB])
                    nc.sync.dma_start(out=a2a_in[j * 128 + 64: (j + 1) * 128, :],
                                      in_=vals1[:, c0:c0 + TSB])
                nc.gpsimd.collective_compute(
                    "AllToAll", mybir.AluOpType.bypass,
                    replica_groups=[list(range(NC))],
                    ins=[a2a_in.opt()], outs=[a2a_out.opt()])
                va = [work.tile([128, TSB], F32, tag=f"va{b}{i}", bufs=1,
                                name=f"va{b}{i}") for i in range(8)]
                va_fr = [work.tile([128, TSB], F32R, tag=f"vafr{b}{i}", bufs=1,
                                   name=f"vafr{b}{i}") for i in range(8)]
                for i in range(8):
                    nc.sync.dma_start(out=va[i][:, :],
                                      in_=a2a_out[i * 128:(i + 1) * 128, :])
                    nc.vector.tensor_copy(out=va_fr[i][:, :], in_=va[i][:, :])
                for mt in range(2):
                    for nb in range(2):
                        op = ps.tile([128, 512], F32, tag="lg", name=f"op{b}{mt}{nb}")
                        for kt in range(8):
                            nc.tensor.matmul(
                                op[:, :],
                                lhsT=va_fr[kt][:, mt * 128:(mt + 1) * 128],
                                rhs=wo_sb[kt][:, nb * 512:(nb + 1) * 512],
                                start=(kt == 0), stop=False)
                        nc.tensor.matmul(
                            op[:, :], lhsT=ones_f[0:1, 0:128],
                            rhs=bo_sb[:, nb * 512:(nb + 1) * 512],
                            start=False, stop=True)
                        ot = work.tile([128, 512], F32, tag="ot", bufs=3,
                                       name=f"ot{b}{mt}{nb}")
                        nc.vector.tensor_copy(out=ot[:, :], in_=op[:, :])
                        nc.sync.dma_start(
                            out=out[b * TSB + mt * 128: b * TSB + (mt + 1) * 128,
                                    nb * 512:(nb + 1) * 512],
                            in_=ot[:, :])

            # ---- schedule ----
            load_block(0)
            load_block(1)
            for i in range(8):
                nc.sync.dma_start(out=wo_sb[i][:, :], in_=woT[i * 128:(i + 1) * 128, :])
            proj_block(0)
            proj_block(1)
            attention(0, 0)
            load_block(2)
            proj_block(2)
            attention(0, 1)
            load_block(3)
            proj_block(3)
            attention(1, 0)
            tail(0)
            attention(1, 1)
            tail(1)

    nc.compile()
    return nc


def _prep_inputs(x, Wqkv, bqkv, Wo, bo):
    x = np.asarray(x, np.float32)
    Wqkv = np.asarray(Wqkv, np.float32)
    bqkv = np.asarray(bqkv, np.float32)
    Wo = np.asarray(Wo, np.float32)
    bo = np.asarray(bo, np.float32)
    xT = np.ascontiguousarray(x.reshape(T, D).T).astype(ml_dtypes.bfloat16)
    woT = np.ascontiguousarray(Wo.T)
    bo2 = np.ascontiguousarray(bo.reshape(1, E))
    in_maps = []
    for c in range(NC):
        h0, h1 = 2 * c, 2 * c + 1
        qk_idx = np.concatenate([
            np.arange(h0 * 192, h0 * 192 + 64),
            np.arange(h1 * 192, h1 * 192 + 64),
            np.arange(h0 * 192 + 64, h0 * 192 + 128),
            np.arange(h1 * 192 + 64, h1 * 192 + 128)])
        v_idx = np.concatenate([
            np.arange(h0 * 192 + 128, h0 * 192 + 192),
            np.arange(h1 * 192 + 128, h1 * 192 + 192)])
        in_maps.append({
            "xT": xT,
            "wqkT": np.ascontiguousarray(Wqkv[qk_idx].T).astype(ml_dtypes.bfloat16),
            "wvT": np.ascontiguousarray(Wqkv[v_idx].T).astype(ml_dtypes.bfloat16),
            "bqk": np.ascontiguousarray(bqkv[qk_idx].reshape(2, 128).T),
            "bv": np.ascontiguousarray(bqkv[v_idx].reshape(1, 128)).astype(
                ml_dtypes.bfloat16),
            "woT": woT,
            "bo": bo2,
        })
    return in_maps


_NC_CACHE = {}


def run(x, Wqkv, bqkv, Wo, bo, trace=False):
    if "nc" not in _NC_CACHE:
        _NC_CACHE["nc"] = build_nc()
    nc = _NC_CACHE["nc"]
    in_maps = _prep_inputs(x, Wqkv, bqkv, Wo, bo)
    res = run_bass_kernel_spmd(nc, in_maps, core_ids=list(range(NC)), trace=trace)
    full = np.empty((T, E), np.float32)
    for c in range(NC):
        o = res.results[c]["out"]
        full[c * 256:(c + 1) * 256] = o[0:256]
        full[2048 + c * 256: 2048 + (c + 1) * 256] = o[256:512]
    return full, res


def kernel(x, Wqkv, bqkv, Wo, bo):
    full, _ = run(x, Wqkv, bqkv, Wo, bo, trace=False)
    return full
